# revision 1
# baseline (speedup 1.0000x reference)
"""ChannelAttention kernel for Trainium2 (8 NeuronCores, batch-parallel).

Reference computation per batch element b (C=64, N=H*W=65536):
    X1 = x[b] viewed [C, N]          (proj_query)
    X2 = x[b] viewed [N, C]          (proj_key -- a reshape, NOT a transpose)
    S  = X1 @ X2                     [C, C]
    P  = softmax(S, axis=-1)
    out[b] = (P @ X1) + X1  =  (P + I) @ X1

Sharding: data-parallel over batch. B=16 -> 2 batches per core on 8 cores.

Per-core dataflow (per batch):
  - x[b] resident in SBUF as 16 column-strips [128, 2048] f32: partition c
    holds X1[c, colhalf0-window], partition 64+c holds X1[c, colhalf1-window].
  - mm1 lhsT tiles: PE-transpose of strip slices [128,128] -> X1^T tiles for
    two n-windows at once (cols 0:64 = window u, cols 64:128 = window u+256).
  - mm1 rhs tiles: X2 contraction-major tiles streamed from HBM with a
    strided access pattern ([128, 32, 64] chunks, 1MB contiguous reads).
  - S accumulates over 512 matmuls in one PSUM tile [64, 64].
  - softmax: DVE row-max (negated) -> ACT exp with fused row-sum ->
    DVE reciprocal -> fused (E * 1/sum) + I.
  - (P+I)^T via PE transpose; replicated to partitions 64:128 via tiny
    SBUF->SBUF DMA so both column-halves of mm2 have aligned operands.
  - mm2: 128 matmuls [64p x 64] @ [64p x 512] -> PSUM -> copy (DVE/ACT
    alternating) into [64, 2048] staging -> 512KB stores to HBM.
"""

import numpy as np

_CACHE = {}

B_FULL = 16
C = 64
N = 65536          # H*W = 256*256
NB = 2             # batches per core
NCORES = 8
NWIN = 256         # 128-col windows per column-half (32768 / 128)
NSTRIP = 16        # strips per batch; strip = [128, 2048]
STRIPW = 2048
VCH = 32           # X2 tiles per V chunk (1 MB per chunk)
NCHUNK = 8         # V chunk pairs per batch (8 * 32 = 256 windows)


def _build(loop_reps=1):
    from contextlib import nullcontext

    import concourse.bacc as bacc
    import concourse.mybir as mybir
    import concourse.tile as tile
    from concourse.masks import make_identity

    f32 = mybir.dt.float32
    Alu = mybir.AluOpType
    Act = mybir.ActivationFunctionType

    nc = bacc.Bacc("TRN2", debug=False)
    xb = nc.dram_tensor("xb", [NB, C * N], f32, kind="ExternalInput").ap()
    ob = nc.dram_tensor("ob", [NB, C * N], f32, kind="ExternalOutput").ap()

    with tile.TileContext(nc) as tc:
        with (
            tc.tile_pool(name="consts", bufs=1) as consts,
            tc.tile_pool(name="H", bufs=NSTRIP) as hpool,
            tc.tile_pool(name="V", bufs=3) as vpool,
            tc.tile_pool(name="TOs", bufs=6) as topool,
            tc.tile_pool(name="stage", bufs=2) as stpool,
            tc.tile_pool(name="soft", bufs=2) as softpool,
            tc.tile_pool(name="psT", bufs=3, space="PSUM") as psT,
            tc.tile_pool(name="psS", bufs=1, space="PSUM") as psS,
            tc.tile_pool(name="psO", bufs=2, space="PSUM") as psO,
            tc.tile_pool(name="psP", bufs=1, space="PSUM") as psP,
        ):
            ident = consts.tile([128, 128], f32)
            make_identity(nc, ident[:])

            loop_cm = (
                tc.For_i(0, loop_reps, 1) if loop_reps > 1 else nullcontext()
            )
            with loop_cm:
              for b in range(NB):
                x1 = xb[b].rearrange("(c n) -> c n", c=C)      # [64, 65536]
                o1 = ob[b].rearrange("(c n) -> c n", c=C)

                # ---- load phase: interleave H strips and V chunks ----
                # Each strip is loaded by two 64-partition DMAs placed on the
                # two HWDGE rings (sync/scalar): partitions 0-63 hit the even
                # SBUF AXI ports and 64-127 the odd ones, so the concurrent
                # pair covers all 16 ports.
                strips = []
                vtiles = []
                for j in range(NCHUNK):
                    for k in (2 * j, 2 * j + 1):
                        st = hpool.tile([128, STRIPW], f32, tag="H")
                        nc.sync.dma_start(
                            st[0:64, :], x1[:, k * STRIPW:(k + 1) * STRIPW]
                        )
                        nc.scalar.dma_start(
                            st[64:128, :],
                            x1[:, 32768 + k * STRIPW: 32768 + (k + 1) * STRIPW],
                        )
                        strips.append(st)
                    # V chunk pair: tiles t in [32j, 32j+32) and [256+32j, ...)
                    # interleaved as [128, tl, half, c] so window u's matmul
                    # can take rhs = [U_u | U_{u+256}] as one [128, 128] slice.
                    vt = vpool.tile([128, VCH, 2, C], f32, tag="V")
                    for half in range(2):
                        t0 = 256 * half + VCH * j
                        src = xb[b][t0 * 8192:(t0 + VCH) * 8192].rearrange(
                            "(t p c) -> p t c", p=128, c=C
                        )
                        nc.sync.dma_start(vt[:, :, half, :], src)
                    vtiles.append(vt)

                # ---- mm1: S = X1 @ X2, accumulated over 512 tiles ----
                # One N=128 matmul per window: lhsT = [T_u | T_{u+256}]
                # (from one transpose), rhs = [U_u | U_{u+256}].  The two
                # diagonal 64x64 blocks of the [128, 128] accumulator hold
                # the real contributions; off-diagonal blocks are ignored.
                # PE stream is software-pipelined: transposes for pair p+SKEW
                # are emitted before the matmuls of pair p, so the PE never
                # waits on the PSUM->SBUF copy chain.
                SKEW = 2
                s_ps = psS.tile([128, 128], f32, tag="S")
                npairs = NWIN // 2               # 128 window pairs
                to_sbs = {}

                def emit_transpose(tp):
                    u0 = 2 * tp
                    to_ps = psT.tile([128, 2, 128], f32, tag="TO")
                    for q in range(2):
                        u = u0 + q
                        st = strips[u // 16]
                        ti = st[:, (u % 16) * 128:(u % 16) * 128 + 128]
                        nc.tensor.transpose(to_ps[:, q, :], ti, ident[:])
                    to_sb = topool.tile([128, 2, 128], f32, tag="TOs")
                    if tp % 2 == 0:
                        nc.scalar.copy(to_sb[:], to_ps[:])
                    else:
                        nc.vector.tensor_copy(to_sb[:], to_ps[:])
                    to_sbs[tp] = to_sb

                for tp in range(SKEW):
                    emit_transpose(tp)
                for tp in range(npairs):
                    if tp + SKEW < npairs:
                        emit_transpose(tp + SKEW)
                    to_sb = to_sbs.pop(tp)
                    for q in range(2):
                        u = 2 * tp + q
                        j, tl = u // VCH, u % VCH
                        nc.tensor.matmul(
                            s_ps[:], to_sb[:, q, :],
                            vtiles[j][:, tl, :, :],
                            start=(u == 0), stop=(u == NWIN - 1),
                        )

                # ---- S = UL + LR (diagonal blocks of the accumulator) ----
                s_sb = softpool.tile([128, 128], f32, tag="Ssb")
                nc.vector.tensor_copy(s_sb[:], s_ps[:])
                s_fix = softpool.tile([64, 64], f32, tag="Sfix")
                nc.sync.dma_start(s_fix[:], s_sb[64:128, 64:128])
                s2_sb = softpool.tile([64, 64], f32, tag="S2")
                nc.vector.tensor_add(s2_sb[:], s_sb[0:64, 0:64], s_fix[:])

                # ---- softmax + (P + I), transposed ----
                nmx = softpool.tile([64, 1], f32, tag="nmx")
                nc.vector.tensor_reduce(
                    nmx[:], s2_sb[:], axis=mybir.AxisListType.X, op=Alu.max,
                    negate=True,
                )
                esum = softpool.tile([64, 1], f32, tag="esum")
                e_sb = softpool.tile([64, 64], f32, tag="E")
                nc.scalar.activation(
                    e_sb[:], s2_sb[:], Act.Exp, bias=nmx[:, 0:1], scale=1.0,
                    accum_out=esum[:],
                )
                rcp = softpool.tile([64, 1], f32, tag="rcp")
                nc.vector.reciprocal(rcp[:], esum[:])
                pi_sb = softpool.tile([64, 64], f32, tag="PI")
                # PI = (E * 1/sum) + I
                nc.vector.scalar_tensor_tensor(
                    pi_sb[:], e_sb[:], rcp[:, 0:1], ident[0:64, 0:64],
                    Alu.mult, Alu.add,
                )
                pit_ps = psP.tile([64, 64], f32, tag="PIT")
                nc.tensor.transpose(pit_ps[:], pi_sb[:], ident[0:64, 0:64])
                pit = softpool.tile([128, 64], f32, tag="PITb")
                nc.vector.tensor_copy(pit[0:64, :], pit_ps[:])
                nc.sync.dma_start(pit[64:128, :], pit[0:64, :])

                # ---- mm2: out = (P+I) @ X1, 128 windows of 512 cols ----
                # Output windows packed two-deep across PSUM/SBUF partition
                # halves (tile_position col groups) so stores run at full
                # 128-partition port width and mm2 matmuls pair up on the
                # two array column halves.
                for half in range(2):
                    lhs = pit[64 * half:64 * half + 64, :]
                    for g in range(8):            # groups of 8 windows (4096)
                        stg = stpool.tile([128, 4, 512], f32, tag="stage")
                        for hb in range(2):
                            for wi in range(4):
                                w = g * 8 + hb * 4 + wi
                                st = strips[w // 4]
                                rhs = st[64 * half:64 * half + 64,
                                         (w % 4) * 512:(w % 4) * 512 + 512]
                                o_ps = psO.tile([128, 512], f32, tag="O")
                                nc.tensor.matmul(
                                    o_ps[64 * hb:64 * hb + 64, :], lhs, rhs,
                                    start=True, stop=True,
                                )
                                if w % 2 == 0:
                                    nc.vector.tensor_copy(
                                        stg[64 * hb:64 * hb + 64, wi, :],
                                        o_ps[64 * hb:64 * hb + 64, :],
                                    )
                                else:
                                    nc.scalar.copy(
                                        stg[64 * hb:64 * hb + 64, wi, :],
                                        o_ps[64 * hb:64 * hb + 64, :],
                                    )
                        off = 32768 * half + g * 4096
                        nc.scalar.dma_start(
                            o1[:, off:off + 2048],
                            stg[0:64].rearrange("p a b -> p (a b)"),
                        )
                        nc.sync.dma_start(
                            o1[:, off + 2048:off + 4096],
                            stg[64:128].rearrange("p a b -> p (a b)"),
                        )

    nc.compile()
    return nc


def kernel(x: np.ndarray) -> np.ndarray:
    from concourse.bass_utils import run_bass_kernel_spmd

    if "nc" not in _CACHE:
        _CACHE["nc"] = _build()
    nc = _CACHE["nc"]

    x = np.ascontiguousarray(x, dtype=np.float32)
    B, Cc, H, W = x.shape
    xflat = x.reshape(B, Cc * H * W)
    in_maps = [
        {"xb": xflat[NB * i:NB * (i + 1)]} for i in range(NCORES)
    ]
    res = run_bass_kernel_spmd(nc, in_maps, core_ids=list(range(NCORES)))
    out = np.empty_like(xflat)
    for i in range(NCORES):
        out[NB * i:NB * (i + 1)] = res.results[i]["ob"]
    return out.reshape(B, Cc, H, W)



# revision 2
# speedup vs baseline: 24.3463x; 24.3463x over previous
"""ChannelAttention kernel v2 for Trainium2 (8 NeuronCores, batch-parallel).

Reference computation per batch element b (C=64, N=H*W=65536):
    X1 = x[b] viewed [C, N]          (proj_query)
    X2 = x[b] viewed [N, C]          (proj_key -- a reshape, NOT a transpose)
    S  = X1 @ X2                     [C, C]
    P  = softmax(S, axis=-1)
    out[b] = (P @ X1) + X1  =  (P + I) @ X1

v3 = v2 but with mm1 (and its transposes) in exact fp32: S logits are
bit-accurate, so the softmax sees no f32r noise; only mm2 runs f32r (its
error is a tiny fraction of the convex-combination output) and stores are
bf16. The kernel is DMA-bound, so the extra PE cycles for fp32 mm1 are
expected to hide under the DMA time.
"""

import numpy as np

_CACHE = {}

B_FULL = 16
C = 64
N = 65536          # H*W = 256*256
NB = 2             # batches per core
NCORES = 8
NWIN = 256         # 128-col windows per column-half (32768 / 128)
NSTRIP = 16        # strips per batch; strip = [128, 2048]
STRIPW = 2048
VCH = 32           # X2 tiles per V chunk (1 MB per chunk)
NCHUNK = 8         # V chunk pairs per batch (8 * 32 = 256 windows)


def _build(loop_reps=1):
    from contextlib import nullcontext

    import concourse.bacc as bacc
    import concourse.mybir as mybir
    import concourse.tile as tile
    from concourse.masks import make_identity

    f32 = mybir.dt.float32
    f32r = mybir.dt.float32r
    bf16 = mybir.dt.bfloat16
    Alu = mybir.AluOpType
    Act = mybir.ActivationFunctionType

    nc = bacc.Bacc("TRN2", debug=False)
    xb = nc.dram_tensor("xb", [NB, C * N], f32r, kind="ExternalInput").ap()
    ob = nc.dram_tensor("ob", [NB, C * N], bf16, kind="ExternalOutput").ap()

    with tile.TileContext(nc) as tc:
        with (
            tc.tile_pool(name="consts", bufs=1) as consts,
            tc.tile_pool(name="H", bufs=NSTRIP) as hpool,
            tc.tile_pool(name="V", bufs=3) as vpool,
            tc.tile_pool(name="TOs", bufs=6) as topool,
            tc.tile_pool(name="stage", bufs=2) as stpool,
            tc.tile_pool(name="soft", bufs=2) as softpool,
            tc.tile_pool(name="psT", bufs=3, space="PSUM") as psT,
            tc.tile_pool(name="psS", bufs=1, space="PSUM") as psS,
            tc.tile_pool(name="psO", bufs=2, space="PSUM") as psO,
            tc.tile_pool(name="psP", bufs=1, space="PSUM") as psP,
        ):
            identf = consts.tile([128, 128], f32)
            make_identity(nc, identf[:])
            # f32r zero block via DMA cast (engine ops cannot emit f32r)
            zerof = consts.tile([64, 64], f32)
            nc.vector.memset(zerof[:], 0.0)
            zeror = consts.tile([64, 64], f32r)
            nc.sync.dma_start(zeror[:], zerof[:].bitcast(f32r))

            loop_cm = (
                tc.For_i(0, loop_reps, 1) if loop_reps > 1 else nullcontext()
            )
            with loop_cm:
              for b in range(NB):
                x1 = xb[b].rearrange("(c n) -> c n", c=C)      # [64, 65536]
                o1 = ob[b].rearrange("(c n) -> c n", c=C)

                # ---- load phase: interleave H strips and V chunks ----
                strips = []
                vtiles = []
                for j in range(NCHUNK):
                    for k in (2 * j, 2 * j + 1):
                        st = hpool.tile([128, STRIPW], f32r, tag="H")
                        nc.sync.dma_start(
                            st[0:64, :], x1[:, k * STRIPW:(k + 1) * STRIPW]
                        )
                        nc.scalar.dma_start(
                            st[64:128, :],
                            x1[:, 32768 + k * STRIPW: 32768 + (k + 1) * STRIPW],
                        )
                        strips.append(st)
                    vt = vpool.tile([128, VCH, 2, C], f32r, tag="V")
                    for half in range(2):
                        t0 = 256 * half + VCH * j
                        src = xb[b][t0 * 8192:(t0 + VCH) * 8192].rearrange(
                            "(t p c) -> p t c", p=128, c=C
                        )
                        nc.sync.dma_start(vt[:, :, half, :], src)
                    vtiles.append(vt)

                # ---- mm1: S = X1 @ X2, accumulated over 512 tiles ----
                # Exact fp32: lhsT = [T_u | T_{u+256}] from one fp32 PE
                # transpose, rhs = [U_u | U_{u+256}] (f32r-typed V tiles read
                # through a bitcast-to-f32 view, which the verifier allows).
                SKEW = 2
                s_ps = psS.tile([128, 128], f32, tag="S")
                npairs = NWIN // 2               # 128 window pairs
                to_sbs = {}

                def emit_transpose(tp):
                    u0 = 2 * tp
                    to_ps = psT.tile([128, 2, 128], f32, tag="TO")
                    for q in range(2):
                        u = u0 + q
                        st = strips[u // 16]
                        ti = st[:, (u % 16) * 128:(u % 16) * 128 + 128]
                        nc.tensor.transpose(
                            to_ps[:, q, :], ti.bitcast(f32), identf[:]
                        )
                    to_sb = topool.tile([128, 2, 128], f32, tag="TOs")
                    if tp % 2 == 0:
                        nc.scalar.copy(to_sb[:], to_ps[:])
                    else:
                        nc.vector.tensor_copy(to_sb[:], to_ps[:])
                    to_sbs[tp] = to_sb

                for tp in range(SKEW):
                    emit_transpose(tp)
                for tp in range(npairs):
                    if tp + SKEW < npairs:
                        emit_transpose(tp + SKEW)
                    to_sb = to_sbs.pop(tp)
                    for q in range(2):
                        u = 2 * tp + q
                        j, tl = u // VCH, u % VCH
                        nc.tensor.matmul(
                            s_ps[:], to_sb[:, q, :],
                            vtiles[j][:, tl, :, :].bitcast(f32),
                            start=(u == 0), stop=(u == NWIN - 1),
                        )

                # ---- S = UL + LR (diagonal blocks of the accumulator) ----
                s_sb = softpool.tile([128, 128], f32, tag="Ssb")
                nc.vector.tensor_copy(s_sb[:], s_ps[:])
                s_fix = softpool.tile([64, 64], f32, tag="Sfix")
                nc.sync.dma_start(s_fix[:], s_sb[64:128, 64:128])
                s2_sb = softpool.tile([64, 64], f32, tag="S2")
                nc.vector.tensor_add(s2_sb[:], s_sb[0:64, 0:64], s_fix[:])

                # ---- softmax + (P + I), transposed ----
                nmx = softpool.tile([64, 1], f32, tag="nmx")
                nc.vector.tensor_reduce(
                    nmx[:], s2_sb[:], axis=mybir.AxisListType.X, op=Alu.max,
                    negate=True,
                )
                esum = softpool.tile([64, 1], f32, tag="esum")
                e_sb = softpool.tile([64, 64], f32, tag="E")
                nc.scalar.activation(
                    e_sb[:], s2_sb[:], Act.Exp, bias=nmx[:, 0:1], scale=1.0,
                    accum_out=esum[:],
                )
                rcp = softpool.tile([64, 1], f32, tag="rcp")
                nc.vector.reciprocal(rcp[:], esum[:])
                pi_sb = softpool.tile([64, 64], f32, tag="PI")
                # PI = (E * 1/sum) + I
                nc.vector.scalar_tensor_tensor(
                    pi_sb[:], e_sb[:], rcp[:, 0:1], identf[0:64, 0:64],
                    Alu.mult, Alu.add,
                )
                # (P+I)^T via plain fp32 transpose, then assemble the f32r
                # block-diagonal lhsT [128,128]: diag blocks = (P+I)^T, rest
                # zero. All f32r writes are DMA casts (engine ops cannot emit
                # f32r), so mm2 runs full-width with no tile_position.
                pit_ps = psP.tile([64, 64], f32, tag="PIT")
                nc.tensor.transpose(pit_ps[:], pi_sb[:], identf[0:64, 0:64])
                pit = softpool.tile([64, 64], f32, tag="PITb")
                nc.vector.tensor_copy(pit[:], pit_ps[:])
                bd = softpool.tile([128, 128], f32r, tag="BD")
                nc.sync.dma_start(
                    bd[0:64, 0:64], pit[:].bitcast(f32r)
                )
                nc.sync.dma_start(
                    bd[0:64, 64:128], zeror[0:64, 0:64]
                )
                nc.scalar.dma_start(
                    bd[64:128, 64:128], pit[:].bitcast(f32r)
                )
                nc.scalar.dma_start(
                    bd[64:128, 0:64], zeror[0:64, 0:64]
                )

                # ---- mm2: out = (P+I) @ X1, one full-width call per 512-col
                # block (both column-halves at once); strip-major so strips
                # free progressively; stores are bf16.
                for k in range(NSTRIP):
                    st = strips[k]
                    stg = stpool.tile([128, 4, 512], bf16, tag="stage")
                    for j in range(4):
                        o_ps = psO.tile([128, 512], f32, tag="O")
                        nc.tensor.matmul(
                            o_ps[:], bd[:], st[:, j * 512:(j + 1) * 512],
                            start=True, stop=True,
                        )
                        if j % 2 == 0:
                            nc.vector.tensor_copy(stg[:, j, :], o_ps[:])
                        else:
                            nc.scalar.copy(stg[:, j, :], o_ps[:])
                    nc.scalar.dma_start(
                        o1[:, k * 2048:(k + 1) * 2048],
                        stg[0:64].rearrange("p a b -> p (a b)"),
                    )
                    nc.sync.dma_start(
                        o1[:, 32768 + k * 2048:32768 + (k + 1) * 2048],
                        stg[64:128].rearrange("p a b -> p (a b)"),
                    )

    nc.compile()
    return nc


def kernel(x: np.ndarray) -> np.ndarray:
    from concourse.bass_utils import run_bass_kernel_spmd

    if "nc" not in _CACHE:
        _CACHE["nc"] = _build()
    nc = _CACHE["nc"]

    x = np.ascontiguousarray(x, dtype=np.float32)
    B, Cc, H, W = x.shape
    xflat = x.reshape(B, Cc * H * W)
    in_maps = [
        {"xb": xflat[NB * i:NB * (i + 1)]} for i in range(NCORES)
    ]
    res = run_bass_kernel_spmd(nc, in_maps, core_ids=list(range(NCORES)))
    out = np.empty_like(xflat)
    for i in range(NCORES):
        out[NB * i:NB * (i + 1)] = np.asarray(
            res.results[i]["ob"]
        ).astype(np.float32)
    return out.reshape(B, Cc, H, W)
